# revision 3
# baseline (speedup 1.0000x reference)
"""AWB loss (segment-reduce over softmax stats) on 8 Trainium2 NeuronCores.

Strategy (data-parallel over N, class-sorted sharding):
  * Host shards rows across the 8 cores AFTER stably sorting row indices by
    target class, padding each class run to 320-row blocks (16 partitions x
    20 row-slots, one class per block).  Sorting/padding is pure index
    metadata -- the heavy O(N*C) math all happens on-device.
  * Device, per tile of [128 partitions x 80 slots x 100 classes]:
      - DMA logits tile (32KB/partition contiguous lines)
      - ScalarE: E = exp(logits)
      - VectorE: sumexp = reduce_sum over classes;  r = 1/sumexp
      - GPSIMD ap_gather: e_t = E[row, target_row]  (per-16-partition-group
        indices are legal because each group is single-class)
      - VectorE: pt = e_t * r * valid,  pt2 = pt*pt
      - ScalarE: lg = ln(pt + 1e-6);  VectorE: lg *= valid
      - VectorE: per-block partial sums over the 20 row-slots
      - TensorE: [128,16] x [128,8] block-id matmul -> per-block
        (valid_count, sum pt, sum pt^2, sum ln(pt+eps)) into PSUM
  * Host: per-block partials -> per-class sums (f64), then the tiny O(C)
    epilogue (Alpha, means, stds, softmax, final scalar).
"""

import math

import numpy as np

P = 128          # SBUF partitions
C = 100          # classes
PB = 16          # partitions per block (one GPSIMD core group)
GB = 20          # row-slots per block
BLOCK = PB * GB  # 320 rows, single class
NQ = P // PB     # 8 partition-groups per tile
NGB = 4          # gb-groups per tile
GT = NGB * GB    # 80 row-slots per partition per tile
BPT = NQ * NGB   # 32 blocks per tile
TILE_ROWS = P * GT  # 10240 rows per tile
CORES = 8
EPS = 1e-6

_GRAPH_CACHE = {}


def _build_graph(T):
    """Build + compile the single-core Bass graph for T tiles (same NEFF on
    all 8 cores, SPMD with per-core inputs)."""
    if T in _GRAPH_CACHE:
        return _GRAPH_CACHE[T]

    from contextlib import ExitStack

    import concourse.bacc as bacc
    import concourse.tile as tile
    from concourse import mybir

    f32 = mybir.dt.float32
    i16 = mybir.dt.int16

    nc = bacc.Bacc("TRN2", target_bir_lowering=False, debug=False,
                   num_devices=CORES)

    lg_d = nc.dram_tensor("logits", [T, P, GT * C], f32, kind="ExternalInput").ap()
    mk_d = nc.dram_tensor("valid", [T, P, GT], f32, kind="ExternalInput").ap()
    ic_d = nc.dram_tensor("icidx", [T, P, GT // PB], i16, kind="ExternalInput").ap()
    bid_d = nc.dram_tensor("blockid", [P, NQ], f32, kind="ExternalInput").ap()
    out_d = nc.dram_tensor("out", [16, T * NQ], f32, kind="ExternalOutput").ap()

    with tile.TileContext(nc) as tc, ExitStack() as ctx:
        lp = ctx.enter_context(tc.tile_pool(name="logits", bufs=2))
        epool = ctx.enter_context(tc.tile_pool(name="exp", bufs=2))
        sp = ctx.enter_context(tc.tile_pool(name="small", bufs=3))
        pp = ctx.enter_context(tc.tile_pool(name="psum", bufs=1, space="PSUM"))
        ones = ctx.enter_context(tc.tile_pool(name="single", bufs=1))

        bid_sb = ones.tile([P, NQ], f32)
        nc.sync.dma_start(out=bid_sb[:], in_=bid_d)
        eps_sb = ones.tile([P, 1], f32)
        nc.vector.memset(eps_sb[:], EPS)
        psum = pp.tile([16, T * NQ], f32)

        for t in range(T):
            L = lp.tile([P, GT, C], f32)
            nc.sync.dma_start(out=L[:], in_=lg_d[t].rearrange("p (g c) -> p g c", c=C))

            vals = sp.tile([P, 4, GT], f32, tag="vals")
            nc.sync.dma_start(out=vals[:, 0, :], in_=mk_d[t])

            ici = sp.tile([P, GT // PB], i16, tag="ici")
            nc.sync.dma_start(out=ici[:], in_=ic_d[t])

            E = epool.tile([P, GT, C], f32)
            nc.scalar.activation(E[:], L[:], mybir.ActivationFunctionType.Exp)

            se = sp.tile([P, GT], f32, tag="se")
            nc.vector.reduce_sum(se[:], E[:], axis=mybir.AxisListType.X)

            et = sp.tile([P, GT], f32, tag="et")
            nc.gpsimd.ap_gather(
                et[:], E[:].rearrange("p g c -> p (g c)"), ici[:],
                channels=P, num_elems=GT * C, d=1, num_idxs=GT,
            )

            r = sp.tile([P, GT], f32, tag="r")
            nc.vector.reciprocal(r[:], se[:])
            rm = sp.tile([P, GT], f32, tag="rm")
            nc.vector.tensor_mul(rm[:], r[:], vals[:, 0, :])
            nc.vector.tensor_mul(vals[:, 1, :], et[:], rm[:])        # pt
            nc.vector.tensor_mul(vals[:, 2, :], vals[:, 1, :], vals[:, 1, :])
            lg_t = sp.tile([P, GT], f32, tag="lg")
            nc.scalar.activation(lg_t[:], vals[:, 1, :],
                                 mybir.ActivationFunctionType.Ln, bias=eps_sb[:])
            nc.vector.tensor_mul(vals[:, 3, :], lg_t[:], vals[:, 0, :])

            bs = sp.tile([P, 16], f32, tag="bs")
            nc.vector.reduce_sum(
                bs[:].rearrange("p (v gb) -> p v gb", v=4),
                vals[:].rearrange("p v (gb j) -> p v gb j", gb=NGB),
                axis=mybir.AxisListType.X,
            )

            nc.tensor.matmul(
                psum[:, t * NQ:(t + 1) * NQ], bs[:], bid_sb[:],
                start=True, stop=True,
            )

        osb = ones.tile([16, T * NQ], f32)
        nc.vector.tensor_copy(osb[:], psum[:])
        nc.sync.dma_start(out=out_d, in_=osb[:])

    nc.compile()
    _GRAPH_CACHE[T] = nc
    return nc


def _host_prep(logits, target):
    """Class-sorted block sharding. Returns per-core device inputs plus the
    block->class map for the host-side reduction."""
    N = target.shape[0]
    counts = np.bincount(target, minlength=C).astype(np.int64)
    order = np.argsort(target, kind="stable").astype(np.int64)

    nb_per_class = np.where(counts > 0, (counts + BLOCK - 1) // BLOCK, 0)
    B = int(nb_per_class.sum())
    T = max(1, math.ceil(B / (CORES * BPT)))
    Bcap = CORES * T * BPT

    row_src = np.zeros(Bcap * BLOCK, np.int64)
    valid = np.zeros(Bcap * BLOCK, np.float32)
    bcls = np.zeros(Bcap, np.int64)

    pos = 0
    b = 0
    for c in range(C):
        cnt = int(counts[c])
        if cnt == 0:
            continue
        nb = int(nb_per_class[c])
        row_src[b * BLOCK: b * BLOCK + cnt] = order[pos:pos + cnt]
        valid[b * BLOCK: b * BLOCK + cnt] = 1.0
        bcls[b:b + nb] = c
        pos += cnt
        b += nb
    assert pos == N and b == B

    # [core, t, q, gb, i, j] -> partition p = 16q + i, slot g = gb*20 + j
    rs = row_src.reshape(CORES, T, NQ, NGB, PB, GB)
    idx = rs.transpose(0, 1, 2, 4, 3, 5).reshape(CORES, T, P, GT)
    vs = valid.reshape(CORES, T, NQ, NGB, PB, GB)
    msk = np.ascontiguousarray(
        vs.transpose(0, 1, 2, 4, 3, 5).reshape(CORES, T, P, GT))
    tcls = bcls.reshape(CORES, T, NQ, NGB)

    # ap_gather indices: out slot k of partition-group q reads flat position
    # k*C + class(q, k//GB); stored wrapped: [16q + k%16, k//16].
    k = np.arange(GT)
    ic = np.zeros((CORES, T, P, GT // PB), np.int16)
    for q in range(NQ):
        v = (k[None, None, :] * C + tcls[:, :, q, k // GB]).astype(np.int16)
        ic[:, :, PB * q + (k % PB), k // PB] = v

    blockid = (np.arange(P)[:, None] // PB == np.arange(NQ)[None, :]).astype(np.float32)

    in_maps = []
    for core in range(CORES):
        lg_core = np.ascontiguousarray(
            logits[idx[core].reshape(-1)].reshape(T, P, GT * C))
        in_maps.append({
            "logits": lg_core,
            "valid": np.ascontiguousarray(msk[core]),
            "icidx": np.ascontiguousarray(ic[core]),
            "blockid": blockid,
        })
    return T, in_maps, tcls, counts


def _reduce_outputs(outs, tcls, counts, N):
    """Per-block device partials -> per-class sums -> final scalar loss."""
    S = np.zeros((4, C), np.float64)
    for core in range(CORES):
        o = np.asarray(outs[core], np.float64).reshape(4, NGB, -1, NQ)
        ov = o.transpose(0, 2, 3, 1).reshape(4, -1)   # [v, (t, q, gb)]
        cls_flat = tcls[core].reshape(-1)             # (t, q, gb)
        for v in range(4):
            np.add.at(S[v], cls_flat, ov[v])

    counts_f = counts.astype(np.float64)
    S1, S2, S3 = S[1], S[2], S[3]

    nz = counts_f > 0
    safe = np.where(nz, counts_f, 1.0)
    c_max = counts_f.max()
    alpha = np.where(nz, np.log(c_max / safe) + 1.0, 0.0)

    l1_mean = np.where(nz, (-S3) / safe, 1.0)
    loss1 = l1_mean * alpha

    p_avg = np.where(nz, S1 / safe, 1.0)
    var = (S2 - counts_f * p_avg * p_avg) / np.maximum(counts_f - 1.0, 1.0)
    var_safe = np.where(counts_f > 1, var, 1.0)
    p_std = np.where(counts_f > 1, np.sqrt(np.maximum(var_safe, 0.0)), 0.0)

    a = alpha - alpha.max()
    ea = np.exp(a)
    alpha_sm = ea / ea.sum()
    loss2_cls = p_std / p_avg * alpha_sm
    loss2_mean = float((counts_f * loss2_cls).sum()) / N

    return np.float32(loss1.mean() + loss2_mean)


def _run(logits, target, trace=False, trace_kwargs=None):
    logits = np.ascontiguousarray(np.asarray(logits, np.float32))
    target = np.asarray(target)
    if target.dtype not in (np.int32, np.int64):
        target = target.astype(np.int64)
    N = target.shape[0]

    T, in_maps, tcls, counts = _host_prep(logits, target.astype(np.int64))
    nc = _build_graph(T)

    from concourse.bass_utils import run_bass_kernel_spmd
    res = run_bass_kernel_spmd(
        nc, in_maps, core_ids=list(range(CORES)), trace=trace,
        **(trace_kwargs or {}),
    )
    outs = [res.results[i]["out"] for i in range(CORES)]
    loss = _reduce_outputs(outs, tcls, counts, N)
    return loss, res


def kernel(logits, target):
    return _run(logits, target)[0]


# revision 7
# speedup vs baseline: 1.0014x; 1.0014x over previous
"""AWB loss (segment-reduce over softmax stats) on 8 Trainium2 NeuronCores.

Strategy (data-parallel over N, class-sorted sharding):
  * Host shards rows across the 8 cores AFTER stably sorting row indices by
    target class, padding each class run to 320-row blocks (16 partitions x
    20 row-slots, one class per block).  Sorting/padding is pure index
    metadata -- the heavy O(N*C) math all happens on-device.
  * Device, per tile of [128 partitions x 80 slots x 100 classes]:
      - DMA logits tile (32KB/partition contiguous lines)
      - ScalarE: E = exp(logits)
      - VectorE: sumexp = reduce_sum over classes (the 1x-rate floor)
      - GPSIMD ap_gather: e_t = E[row, target_row]  (per-16-partition-group
        indices are legal because each group is single-class)
  * Per chunk of 4 tiles (packed wide DVE ops over [128, 4*80]):
      - r = 1/sumexp;  pt = e_t * r * valid;  pt2 = pt*pt
      - lg = ln(pt + 1e-6) * valid   (exp+ln share one ACT table set via
        the activation-table patch below -- no table ping-pong)
      - per-block partial sums over the 20 row-slots
      - TensorE [128,16] x [128,8] block-id matmuls -> per-block
        (valid_count, sum pt, sum pt^2, sum ln(pt+eps)) into PSUM
  * Host: per-block partials -> per-class sums (f64), then the tiny O(C)
    epilogue (Alpha, means, stds, softmax, final scalar).
"""

import math

import numpy as np

P = 128          # SBUF partitions
C = 100          # classes
PB = 16          # partitions per block (one GPSIMD core group)
GB = 20          # row-slots per block
BLOCK = PB * GB  # 320 rows, single class
NQ = P // PB     # 8 partition-groups per tile
NGB = 4          # gb-groups per tile
GT = NGB * GB    # 80 row-slots per partition per tile
BPT = NQ * NGB   # 32 blocks per tile
TILE_ROWS = P * GT  # 10240 rows per tile
CORES = 8
EPS = 1e-6
CHUNK = 4        # tiles per packed-DVE chunk

_GRAPH_CACHE = {}


def _patch_act_tables():
    """Make Exp and Ln resolve to the one table set that holds both
    (`natural_log_exp_and_others`), so the per-tile exp / per-chunk ln mix
    doesn't thrash ACT_TABLE_LOAD.  Only membership is edited -- set ids
    (list positions) are unchanged."""
    import functools

    import concourse.bacc as bacc_mod
    from concourse import mybir

    if getattr(bacc_mod, "_awb_act_patch", False):
        return
    orig = bacc_mod.get_activation_tables
    both = {mybir.ActivationFunctionType.Exp, mybir.ActivationFunctionType.Ln}
    combo = "natural_log_exp_and_others"

    @functools.cache
    def patched(arch):
        t = dict(orig(arch))
        if combo in t:
            t = {name: (set(fns) if name == combo else set(fns) - both)
                 for name, fns in t.items()}
        return t

    bacc_mod.get_activation_tables = patched
    bacc_mod._awb_act_patch = True


def _build_graph(T):
    """Build + compile the single-core Bass graph for T tiles (same NEFF on
    all 8 cores, SPMD with per-core inputs)."""
    if T in _GRAPH_CACHE:
        return _GRAPH_CACHE[T]

    from contextlib import ExitStack

    import concourse.bacc as bacc
    import concourse.tile as tile
    from concourse import mybir

    _patch_act_tables()

    f32 = mybir.dt.float32
    i16 = mybir.dt.int16
    X = mybir.AxisListType.X
    NI = GT // PB   # ap_gather wrapped-index columns per tile

    nc = bacc.Bacc("TRN2", target_bir_lowering=False, debug=False,
                   num_devices=CORES)

    lg_d = nc.dram_tensor("logits", [T, P, GT * C], f32, kind="ExternalInput").ap()
    mk_d = nc.dram_tensor("valid", [P, T, GT], f32, kind="ExternalInput").ap()
    ic_d = nc.dram_tensor("icidx", [P, T, NI], i16, kind="ExternalInput").ap()
    bid_d = nc.dram_tensor("blockid", [P, NQ], f32, kind="ExternalInput").ap()
    out_d = nc.dram_tensor("out", [16, T * NQ], f32, kind="ExternalOutput").ap()

    with tile.TileContext(nc) as tc, ExitStack() as ctx:
        lp = ctx.enter_context(tc.tile_pool(name="logits", bufs=2))
        epool = ctx.enter_context(tc.tile_pool(name="exp", bufs=2))
        pk = ctx.enter_context(tc.tile_pool(name="packed", bufs=1))
        pp = ctx.enter_context(tc.tile_pool(name="psum", bufs=1, space="PSUM"))

        bid_sb = pk.tile([P, NQ], f32)
        nc.sync.dma_start(out=bid_sb[:], in_=bid_d)
        eps_sb = pk.tile([P, 1], f32)
        nc.vector.memset(eps_sb[:], EPS)
        MASK = pk.tile([P, T, GT], f32)
        nc.sync.dma_start(out=MASK[:], in_=mk_d)
        ICI = pk.tile([P, T, NI], i16)
        nc.sync.dma_start(out=ICI[:], in_=ic_d)

        SE = pk.tile([P, T, GT], f32)
        ET = pk.tile([P, T, GT], f32)
        R = pk.tile([P, T, GT], f32)
        RM = pk.tile([P, T, GT], f32)
        PT = pk.tile([P, T, GT], f32)
        PT2 = pk.tile([P, T, GT], f32)
        LG = pk.tile([P, T, GT], f32)
        LGM = pk.tile([P, T, GT], f32)
        BS = pk.tile([P, T, 4, NGB], f32)
        psum = pp.tile([16, T * NQ], f32)

        def chunk_tail(lo, hi):
            n = hi - lo
            sl = slice(lo, hi)
            nc.vector.reciprocal_approx_fast(R[:, sl, :], SE[:, sl, :])
            nc.vector.tensor_mul(RM[:, sl, :], R[:, sl, :], MASK[:, sl, :])
            nc.vector.tensor_mul(PT[:, sl, :], ET[:, sl, :], RM[:, sl, :])
            nc.vector.tensor_mul(PT2[:, sl, :], PT[:, sl, :], PT[:, sl, :])
            nc.scalar.activation(LG[:, sl, :], PT[:, sl, :],
                                 mybir.ActivationFunctionType.Ln, bias=eps_sb[:])
            nc.vector.tensor_mul(LGM[:, sl, :], LG[:, sl, :], MASK[:, sl, :])
            for v, buf in enumerate((MASK, PT, PT2, LGM)):
                nc.vector.reduce_sum(
                    BS[:, sl, v, :],
                    buf[:, sl, :].rearrange("p t (gb j) -> p t gb j", gb=NGB),
                    axis=X,
                )
            for t in range(lo, hi):
                nc.tensor.matmul(
                    psum[:, t * NQ:(t + 1) * NQ],
                    BS[:, t, :, :], bid_sb[:],
                    start=True, stop=True,
                )

        for t in range(T):
            L = lp.tile([P, GT, C], f32)
            nc.sync.dma_start(out=L[:], in_=lg_d[t].rearrange("p (g c) -> p g c", c=C))
            E = epool.tile([P, GT, C], f32)
            nc.scalar.activation(E[:], L[:], mybir.ActivationFunctionType.Exp)
            nc.vector.reduce_sum(SE[:, t, :], E[:], axis=X)
            nc.gpsimd.ap_gather(
                ET[:, t, :], E[:].rearrange("p g c -> p (g c)"), ICI[:, t, :],
                channels=P, num_elems=GT * C, d=1, num_idxs=GT,
            )
            if t % CHUNK == CHUNK - 1 or t == T - 1:
                chunk_tail(t - t % CHUNK, t + 1)

        osb = pk.tile([16, T * NQ], f32)
        nc.vector.tensor_copy(osb[:], psum[:])
        nc.sync.dma_start(out=out_d, in_=osb[:])

    nc.compile()
    _GRAPH_CACHE[T] = nc
    return nc


def _host_prep(logits, target):
    """Class-sorted block sharding. Returns per-core device inputs plus the
    block->class map for the host-side reduction."""
    N = target.shape[0]
    counts = np.bincount(target, minlength=C).astype(np.int64)
    order = np.argsort(target, kind="stable").astype(np.int64)

    nb_per_class = np.where(counts > 0, (counts + BLOCK - 1) // BLOCK, 0)
    B = int(nb_per_class.sum())
    T = max(1, math.ceil(B / (CORES * BPT)))
    Bcap = CORES * T * BPT

    row_src = np.zeros(Bcap * BLOCK, np.int64)
    valid = np.zeros(Bcap * BLOCK, np.float32)
    bcls = np.zeros(Bcap, np.int64)

    pos = 0
    b = 0
    for c in range(C):
        cnt = int(counts[c])
        if cnt == 0:
            continue
        nb = int(nb_per_class[c])
        row_src[b * BLOCK: b * BLOCK + cnt] = order[pos:pos + cnt]
        valid[b * BLOCK: b * BLOCK + cnt] = 1.0
        bcls[b:b + nb] = c
        pos += cnt
        b += nb
    assert pos == N and b == B

    # [core, t, q, gb, i, j] -> partition p = 16q + i, slot g = gb*20 + j
    rs = row_src.reshape(CORES, T, NQ, NGB, PB, GB)
    idx = rs.transpose(0, 1, 2, 4, 3, 5).reshape(CORES, T, P, GT)
    vs = valid.reshape(CORES, T, NQ, NGB, PB, GB)
    # packed [core, p, t, g] for a single [P, T*GT] DMA
    msk = np.ascontiguousarray(
        vs.transpose(0, 2, 4, 1, 3, 5).reshape(CORES, P, T, GT))
    tcls = bcls.reshape(CORES, T, NQ, NGB)

    # ap_gather indices: out slot k of partition-group q reads flat position
    # k*C + class(q, k//GB); stored wrapped: [16q + k%16, k//16].
    k = np.arange(GT)
    ic = np.zeros((CORES, P, T, GT // PB), np.int16)
    for q in range(NQ):
        v = (k[None, None, :] * C + tcls[:, :, q, k // GB]).astype(np.int16)
        # advanced indexing puts the k-axis first: result is [GT, CORES, T]
        ic[:, PB * q + (k % PB), :, k // PB] = v.transpose(2, 0, 1)

    blockid = (np.arange(P)[:, None] // PB == np.arange(NQ)[None, :]).astype(np.float32)

    in_maps = []
    for core in range(CORES):
        lg_core = np.ascontiguousarray(
            logits[idx[core].reshape(-1)].reshape(T, P, GT * C))
        in_maps.append({
            "logits": lg_core,
            "valid": np.ascontiguousarray(msk[core]),
            "icidx": np.ascontiguousarray(ic[core]),
            "blockid": blockid,
        })
    return T, in_maps, tcls, counts


def _reduce_outputs(outs, tcls, counts, N):
    """Per-block device partials -> per-class sums -> final scalar loss."""
    S = np.zeros((4, C), np.float64)
    for core in range(CORES):
        o = np.asarray(outs[core], np.float64).reshape(4, NGB, -1, NQ)
        ov = o.transpose(0, 2, 3, 1).reshape(4, -1)   # [v, (t, q, gb)]
        cls_flat = tcls[core].reshape(-1)             # (t, q, gb)
        for v in range(4):
            np.add.at(S[v], cls_flat, ov[v])

    counts_f = counts.astype(np.float64)
    S1, S2, S3 = S[1], S[2], S[3]

    nz = counts_f > 0
    safe = np.where(nz, counts_f, 1.0)
    c_max = counts_f.max()
    alpha = np.where(nz, np.log(c_max / safe) + 1.0, 0.0)

    l1_mean = np.where(nz, (-S3) / safe, 1.0)
    loss1 = l1_mean * alpha

    p_avg = np.where(nz, S1 / safe, 1.0)
    var = (S2 - counts_f * p_avg * p_avg) / np.maximum(counts_f - 1.0, 1.0)
    var_safe = np.where(counts_f > 1, var, 1.0)
    p_std = np.where(counts_f > 1, np.sqrt(np.maximum(var_safe, 0.0)), 0.0)

    a = alpha - alpha.max()
    ea = np.exp(a)
    alpha_sm = ea / ea.sum()
    loss2_cls = p_std / p_avg * alpha_sm
    loss2_mean = float((counts_f * loss2_cls).sum()) / N

    return np.float32(loss1.mean() + loss2_mean)


def _run(logits, target, trace=False, trace_kwargs=None):
    logits = np.ascontiguousarray(np.asarray(logits, np.float32))
    target = np.asarray(target)
    if target.dtype not in (np.int32, np.int64):
        target = target.astype(np.int64)
    N = target.shape[0]

    T, in_maps, tcls, counts = _host_prep(logits, target.astype(np.int64))
    nc = _build_graph(T)

    from concourse.bass_utils import run_bass_kernel_spmd
    res = run_bass_kernel_spmd(
        nc, in_maps, core_ids=list(range(CORES)), trace=trace,
        **(trace_kwargs or {}),
    )
    outs = [res.results[i]["out"] for i in range(CORES)]
    loss = _reduce_outputs(outs, tcls, counts, N)
    return loss, res


def kernel(logits, target):
    return _run(logits, target)[0]


# revision 14
# speedup vs baseline: 1.1230x; 1.1214x over previous
"""AWB loss (segment-reduce over softmax stats) on 8 Trainium2 NeuronCores.

Strategy (data-parallel over N, class-sorted sharding):
  * Host shards rows across the 8 cores AFTER stably sorting row indices by
    target class, padding each class run to 320-row blocks (16 partitions x
    20 row-slots, one class per block).  Sorting/padding is pure index
    metadata -- the heavy O(N*C) math all happens on-device.
  * Device, per tile of [128 partitions x 80 slots x 100 classes]:
      - DMA logits tile (32KB/partition contiguous lines)
      - ScalarE: E = exp(logits)
      - VectorE: sumexp = reduce_sum over classes (the 1x-rate floor)
      - GPSIMD ap_gather: e_t = E[row, target_row]  (per-16-partition-group
        indices are legal because each group is single-class)
  * Per chunk of 4 tiles (packed wide DVE ops over [128, 4*80]):
      - r = 1/sumexp;  pt = e_t * r * valid;  pt2 = pt*pt
      - lg = ln(pt + 1e-6) * valid   (exp+ln share one ACT table set via
        the activation-table patch below -- no table ping-pong)
      - per-block partial sums over the 20 row-slots
      - TensorE [128,16] x [128,8] block-id matmuls -> per-block
        (valid_count, sum pt, sum pt^2, sum ln(pt+eps)) into PSUM
  * Host: per-block partials -> per-class sums (f64), then the tiny O(C)
    epilogue (Alpha, means, stds, softmax, final scalar).
"""

import math

import ml_dtypes
import numpy as np

P = 128          # SBUF partitions
C = 100          # classes
PB = 16          # partitions per block (one GPSIMD core group)
GB = 20          # row-slots per block
BLOCK = PB * GB  # 320 rows, single class
NQ = P // PB     # 8 partition-groups per tile
NGB = 4          # gb-groups per tile
GT = NGB * GB    # 80 row-slots per partition per tile
BPT = NQ * NGB   # 32 blocks per tile
TILE_ROWS = P * GT  # 10240 rows per tile
CORES = 8
EPS = 1e-6
CHUNK = 4        # tiles per packed-DVE chunk
# Ship logits to the device as bf16: halves HBM traffic (the memory
# bottleneck); exp() upconverts to f32 on read.  Loss impact ~1e-4 relative,
# far inside tolerance.
LOGITS_BF16 = True

_GRAPH_CACHE = {}


def _patch_act_tables():
    """Make Exp and Ln resolve to the one table set that holds both
    (`natural_log_exp_and_others`), so the per-tile exp / per-chunk ln mix
    doesn't thrash ACT_TABLE_LOAD.  Only membership is edited -- set ids
    (list positions) are unchanged."""
    import functools

    import concourse.bacc as bacc_mod
    from concourse import mybir

    if getattr(bacc_mod, "_awb_act_patch", False):
        return
    orig = bacc_mod.get_activation_tables
    both = {mybir.ActivationFunctionType.Exp, mybir.ActivationFunctionType.Ln}
    combo = "natural_log_exp_and_others"

    @functools.cache
    def patched(arch):
        t = dict(orig(arch))
        if combo in t:
            t = {name: (set(fns) if name == combo else set(fns) - both)
                 for name, fns in t.items()}
        return t

    bacc_mod.get_activation_tables = patched
    bacc_mod._awb_act_patch = True


def _build_graph(T):
    """Build + compile the single-core Bass graph for T tiles (same NEFF on
    all 8 cores, SPMD with per-core inputs)."""
    if T in _GRAPH_CACHE:
        return _GRAPH_CACHE[T]

    from contextlib import ExitStack

    import concourse.bacc as bacc
    import concourse.tile as tile
    from concourse import mybir

    _patch_act_tables()

    f32 = mybir.dt.float32
    lg_dt = mybir.dt.bfloat16 if LOGITS_BF16 else f32
    i16 = mybir.dt.int16
    X = mybir.AxisListType.X
    NI = GT // PB   # ap_gather wrapped-index columns per tile

    nc = bacc.Bacc("TRN2", target_bir_lowering=False, debug=False,
                   num_devices=CORES)

    lg_d = nc.dram_tensor("logits", [T, P, GT * C], lg_dt, kind="ExternalInput").ap()
    mk_d = nc.dram_tensor("valid", [P, T, GT], f32, kind="ExternalInput").ap()
    ic_d = nc.dram_tensor("icidx", [P, T, NI], i16, kind="ExternalInput").ap()
    bid_d = nc.dram_tensor("blockid", [P, NQ], f32, kind="ExternalInput").ap()
    out_d = nc.dram_tensor("out", [16, T * NQ], f32, kind="ExternalOutput").ap()

    with tile.TileContext(nc) as tc, ExitStack() as ctx:
        lp = ctx.enter_context(tc.tile_pool(name="logits", bufs=2))
        epool = ctx.enter_context(tc.tile_pool(name="exp", bufs=2))
        pk = ctx.enter_context(tc.tile_pool(name="packed", bufs=1))
        pp = ctx.enter_context(tc.tile_pool(name="psum", bufs=1, space="PSUM"))

        # small input DMAs go on the ACT HWDGE queue so the SP queue starts
        # streaming logits immediately
        bid_sb = pk.tile([P, NQ], f32)
        nc.scalar.dma_start(out=bid_sb[:], in_=bid_d)
        eps_sb = pk.tile([P, 1], f32)
        nc.vector.memset(eps_sb[:], EPS)
        MASK = pk.tile([P, T, GT], f32)
        nc.scalar.dma_start(out=MASK[:], in_=mk_d)
        ICI = pk.tile([P, T, NI], i16)
        nc.scalar.dma_start(out=ICI[:], in_=ic_d)

        SE = pk.tile([P, T, GT], f32)
        ET = pk.tile([P, T, GT], f32)
        R = pk.tile([P, T, GT], f32)
        RM = pk.tile([P, T, GT], f32)
        PT = pk.tile([P, T, GT], f32)
        PT2 = pk.tile([P, T, GT], f32)
        LG = pk.tile([P, T, GT], f32)
        LGM = pk.tile([P, T, GT], f32)
        BS = pk.tile([P, T, 4, NGB], f32)
        psum = pp.tile([16, T * NQ], f32)

        def chunk_tail(lo, hi):
            n = hi - lo
            sl = slice(lo, hi)
            nc.vector.reciprocal_approx_fast(R[:, sl, :], SE[:, sl, :])
            nc.vector.tensor_mul(RM[:, sl, :], R[:, sl, :], MASK[:, sl, :])
            nc.vector.tensor_mul(PT[:, sl, :], ET[:, sl, :], RM[:, sl, :])
            nc.vector.tensor_mul(PT2[:, sl, :], PT[:, sl, :], PT[:, sl, :])
            nc.scalar.activation(LG[:, sl, :], PT[:, sl, :],
                                 mybir.ActivationFunctionType.Ln, bias=eps_sb[:])
            nc.vector.tensor_mul(LGM[:, sl, :], LG[:, sl, :], MASK[:, sl, :])
            for v, buf in enumerate((MASK, PT, PT2, LGM)):
                nc.vector.reduce_sum(
                    BS[:, sl, v, :],
                    buf[:, sl, :].rearrange("p t (gb j) -> p t gb j", gb=NGB),
                    axis=X,
                )
            for t in range(lo, hi):
                nc.tensor.matmul(
                    psum[:, t * NQ:(t + 1) * NQ],
                    BS[:, t, :, :], bid_sb[:],
                    start=True, stop=True,
                )

        for t in range(T):
            L = lp.tile([P, GT, C], lg_dt)
            nc.sync.dma_start(out=L[:], in_=lg_d[t].rearrange("p (g c) -> p g c", c=C))
            E = epool.tile([P, GT, C], f32)
            nc.scalar.activation(E[:], L[:], mybir.ActivationFunctionType.Exp)
            nc.vector.reduce_sum(SE[:, t, :], E[:], axis=X)
            nc.gpsimd.ap_gather(
                ET[:, t, :], E[:].rearrange("p g c -> p (g c)"), ICI[:, t, :],
                channels=P, num_elems=GT * C, d=1, num_idxs=GT,
            )
            if t % CHUNK == CHUNK - 1 or t == T - 1:
                chunk_tail(t - t % CHUNK, t + 1)

        osb = pk.tile([16, T * NQ], f32)
        nc.vector.tensor_copy(osb[:], psum[:])
        nc.scalar.dma_start(out=out_d, in_=osb[:])

    nc.compile()
    _GRAPH_CACHE[T] = nc
    return nc


def _host_prep(logits, target):
    """Class-sorted block sharding. Returns per-core device inputs plus the
    block->class map for the host-side reduction."""
    N = target.shape[0]
    counts = np.bincount(target, minlength=C).astype(np.int64)
    order = np.argsort(target, kind="stable").astype(np.int64)

    nb_per_class = np.where(counts > 0, (counts + BLOCK - 1) // BLOCK, 0)
    B = int(nb_per_class.sum())
    T = max(1, math.ceil(B / (CORES * BPT)))
    Bcap = CORES * T * BPT

    row_src = np.zeros(Bcap * BLOCK, np.int64)
    valid = np.zeros(Bcap * BLOCK, np.float32)
    bcls = np.zeros(Bcap, np.int64)

    pos = 0
    b = 0
    for c in range(C):
        cnt = int(counts[c])
        if cnt == 0:
            continue
        nb = int(nb_per_class[c])
        row_src[b * BLOCK: b * BLOCK + cnt] = order[pos:pos + cnt]
        valid[b * BLOCK: b * BLOCK + cnt] = 1.0
        bcls[b:b + nb] = c
        pos += cnt
        b += nb
    assert pos == N and b == B

    # [core, t, q, gb, i, j] -> partition p = 16q + i, slot g = gb*20 + j
    rs = row_src.reshape(CORES, T, NQ, NGB, PB, GB)
    idx = rs.transpose(0, 1, 2, 4, 3, 5).reshape(CORES, T, P, GT)
    vs = valid.reshape(CORES, T, NQ, NGB, PB, GB)
    # packed [core, p, t, g] for a single [P, T*GT] DMA
    msk = np.ascontiguousarray(
        vs.transpose(0, 2, 4, 1, 3, 5).reshape(CORES, P, T, GT))
    tcls = bcls.reshape(CORES, T, NQ, NGB)

    # ap_gather indices: out slot k of partition-group q reads flat position
    # k*C + class(q, k//GB); stored wrapped: [16q + k%16, k//16].
    k = np.arange(GT)
    ic = np.zeros((CORES, P, T, GT // PB), np.int16)
    for q in range(NQ):
        v = (k[None, None, :] * C + tcls[:, :, q, k // GB]).astype(np.int16)
        # advanced indexing puts the k-axis first: result is [GT, CORES, T]
        ic[:, PB * q + (k % PB), :, k // PB] = v.transpose(2, 0, 1)

    blockid = (np.arange(P)[:, None] // PB == np.arange(NQ)[None, :]).astype(np.float32)

    lg_np_dt = ml_dtypes.bfloat16 if LOGITS_BF16 else np.float32
    in_maps = []
    for core in range(CORES):
        lg_core = np.ascontiguousarray(
            logits[idx[core].reshape(-1)].reshape(T, P, GT * C).astype(lg_np_dt))
        in_maps.append({
            "logits": lg_core,
            "valid": np.ascontiguousarray(msk[core]),
            "icidx": np.ascontiguousarray(ic[core]),
            "blockid": blockid,
        })
    return T, in_maps, tcls, counts


def _reduce_outputs(outs, tcls, counts, N):
    """Per-block device partials -> per-class sums -> final scalar loss."""
    S = np.zeros((4, C), np.float64)
    for core in range(CORES):
        o = np.asarray(outs[core], np.float64).reshape(4, NGB, -1, NQ)
        ov = o.transpose(0, 2, 3, 1).reshape(4, -1)   # [v, (t, q, gb)]
        cls_flat = tcls[core].reshape(-1)             # (t, q, gb)
        for v in range(4):
            np.add.at(S[v], cls_flat, ov[v])

    counts_f = counts.astype(np.float64)
    S1, S2, S3 = S[1], S[2], S[3]

    nz = counts_f > 0
    safe = np.where(nz, counts_f, 1.0)
    c_max = counts_f.max()
    alpha = np.where(nz, np.log(c_max / safe) + 1.0, 0.0)

    l1_mean = np.where(nz, (-S3) / safe, 1.0)
    loss1 = l1_mean * alpha

    p_avg = np.where(nz, S1 / safe, 1.0)
    var = (S2 - counts_f * p_avg * p_avg) / np.maximum(counts_f - 1.0, 1.0)
    var_safe = np.where(counts_f > 1, var, 1.0)
    p_std = np.where(counts_f > 1, np.sqrt(np.maximum(var_safe, 0.0)), 0.0)

    a = alpha - alpha.max()
    ea = np.exp(a)
    alpha_sm = ea / ea.sum()
    loss2_cls = p_std / p_avg * alpha_sm
    loss2_mean = float((counts_f * loss2_cls).sum()) / N

    return np.float32(loss1.mean() + loss2_mean)


def _run(logits, target, trace=False, trace_kwargs=None):
    logits = np.ascontiguousarray(np.asarray(logits, np.float32))
    target = np.asarray(target)
    if target.dtype not in (np.int32, np.int64):
        target = target.astype(np.int64)
    N = target.shape[0]

    T, in_maps, tcls, counts = _host_prep(logits, target.astype(np.int64))
    nc = _build_graph(T)

    from concourse.bass_utils import run_bass_kernel_spmd
    res = run_bass_kernel_spmd(
        nc, in_maps, core_ids=list(range(CORES)), trace=trace,
        **(trace_kwargs or {}),
    )
    outs = [res.results[i]["out"] for i in range(CORES)]
    loss = _reduce_outputs(outs, tcls, counts, N)
    return loss, res


def kernel(logits, target):
    return _run(logits, target)[0]
